# revision 15
# baseline (speedup 1.0000x reference)
import numpy as np
import ml_dtypes

from concourse import bass
from concourse import bacc
from concourse import mybir
from concourse.tile import TileContext
from concourse import bass_utils

# Gemma2 sliding-window attention — hardcoded problem shapes.
B, S, H = 4, 2048, 2048
NH, NKV, HD = 8, 4, 256
WINDOW = 1024
ROPE_BASE = 10000.0
SCALE = 256.0 ** -0.5

BF16 = mybir.dt.bfloat16
F32 = mybir.dt.float32
NPBF16 = ml_dtypes.bfloat16

N_CORES = 8
NT_H = H // 128          # 16 h-tiles
NT_S = S // 128          # 16 s-tiles
N_SB = S // 512          # 4 s-blocks
QK_D = 1024 + 512        # per-core Q (4 heads) + K (2 kv heads) transposed rows
NT_QK = QK_D // 128      # 12
V_D = 512                # per-core V columns (2 kv heads)
WO_D = 1024              # per-core Wo contraction rows

_CACHE = {}


def _build():
    nc = bacc.Bacc("TRN2", target_bir_lowering=False, debug=False)

    hsT = nc.dram_tensor("hsT", [H, S], BF16, kind="ExternalInput")
    wqk = nc.dram_tensor("wqk", [H, QK_D], BF16, kind="ExternalInput")
    wv = nc.dram_tensor("wv", [H, V_D], BF16, kind="ExternalInput")
    wo = nc.dram_tensor("wo", [WO_D, H], BF16, kind="ExternalInput")
    cosd = nc.dram_tensor("cosd", [128, S], BF16, kind="ExternalInput")
    sind = nc.dram_tensor("sind", [128, S], BF16, kind="ExternalInput")
    diagd = nc.dram_tensor("diagd", [128, 128], BF16, kind="ExternalInput")
    wedged = nc.dram_tensor("wedged", [128, 128], BF16, kind="ExternalInput")
    onesd = nc.dram_tensor("onesd", [128, 128], BF16, kind="ExternalInput")
    biasqkd = nc.dram_tensor("biasqkd", [128, NT_QK], F32, kind="ExternalInput")
    bvbd = nc.dram_tensor("bvbd", [128, V_D], F32, kind="ExternalInput")
    outT = nc.dram_tensor("outT", [H, S], BF16, kind="ExternalOutput")

    with TileContext(nc) as tc:
        # ---------------- persistent SBUF ----------------
        pp = tc.alloc_tile_pool(name="persist", bufs=1)
        cos_t = pp.tile([128, S], BF16, tag="cos")
        sin_t = pp.tile([128, S], BF16, tag="sin")
        diag_t = pp.tile([128, 128], BF16, tag="diag")
        wedge_t = pp.tile([128, 128], BF16, tag="wedge")
        ones_t = pp.tile([128, 128], BF16, tag="ones")
        biasqk_t = pp.tile([128, NT_QK], F32, tag="biasqk")
        bvb_t = pp.tile([128, V_D], F32, tag="bvb")
        qk_t = [pp.tile([128, S], BF16, tag=f"qk{i}", name=f"qk{i}") for i in range(NT_QK)]
        v_t = [pp.tile([128, V_D], BF16, tag=f"v{i}", name=f"v{i}") for i in range(NT_S)]

        # SWDGE (single queue) for tensors consumed by DVE — a HWDGE
        # transfer fans out across HW queues and the first consuming
        # TensorTensor instruction would need one sync-wait per queue,
        # overflowing the ISA wait capacity.
        nc.gpsimd.dma_start(cos_t[:], cosd[:, :])
        nc.gpsimd.dma_start(sin_t[:], sind[:, :])
        nc.gpsimd.dma_start(diag_t[:], diagd[:, :])
        nc.gpsimd.dma_start(wedge_t[:], wedged[:, :])
        nc.gpsimd.dma_start(ones_t[:], onesd[:, :])
        nc.gpsimd.dma_start(biasqk_t[:], biasqkd[:, :])
        nc.gpsimd.dma_start(bvb_t[:], bvbd[:, :])

        # ---------------- load hidden states (transposed) + Wv ----------------
        hsp = tc.alloc_tile_pool(name="hst", bufs=1)
        hst = [hsp.tile([128, S], BF16, tag=f"h{i}", name=f"h{i}") for i in range(NT_H)]
        for i in range(NT_H):
            nc.sync.dma_start(hst[i][:], hsT[i * 128:(i + 1) * 128, :])

        wvp = tc.alloc_tile_pool(name="wvp", bufs=1)
        wv_t = [wvp.tile([128, V_D], BF16, tag=f"wv{i}", name=f"wv{i}") for i in range(NT_H)]
        for i in range(NT_H):
            nc.sync.dma_start(wv_t[i][:], wv[i * 128:(i + 1) * 128, :])

        # ---------------- V = hs @ Wv  (layout [s, d]) ----------------
        psv = tc.alloc_tile_pool(name="psv", bufs=4, space="PSUM")
        for st in range(NT_S):
            ps = psv.tile([128, V_D], F32, tag="psv")
            for ht in range(NT_H):
                nc.tensor.matmul(
                    ps[:],
                    hst[ht][:, st * 128:(st + 1) * 128],
                    wv_t[ht][:],
                    start=(ht == 0),
                    stop=(ht == NT_H - 1),
                )
            nc.vector.tensor_add(v_t[st][:], ps[:], bvb_t[:])
        psv.release()
        wvp.release()

        # ---------------- QK^T = (Wqk)^T @ hs^T  (layout [d, s]) ----------------
        wqkp = tc.alloc_tile_pool(name="wqkp", bufs=1)
        wqk_t = [wqkp.tile([128, QK_D], BF16, tag=f"wqk{i}", name=f"wqk{i}") for i in range(NT_H)]
        for i in range(NT_H):
            nc.sync.dma_start(wqk_t[i][:], wqk[i * 128:(i + 1) * 128, :])

        # tiles 6..15 land in virgin SBUF (no dep on wv release) — do them first
        ht_order = list(range(6, NT_H)) + list(range(6))
        psqk = tc.alloc_tile_pool(name="psqk", bufs=6, space="PSUM")
        for dt in range(NT_QK):
            for sb in range(N_SB):
                ps = psqk.tile([128, 512], F32, tag="psqk")
                for j, ht in enumerate(ht_order):
                    nc.tensor.matmul(
                        ps[:],
                        wqk_t[ht][:, dt * 128:(dt + 1) * 128],
                        hst[ht][:, sb * 512:(sb + 1) * 512],
                        start=(j == 0),
                        stop=(j == NT_H - 1),
                    )
                nc.vector.tensor_scalar_add(
                    qk_t[dt][:, sb * 512:(sb + 1) * 512], ps[:],
                    biasqk_t[:, dt:dt + 1],
                )
        psqk.release()
        wqkp.release()
        hsp.release()

        # ---------------- attn^T output tiles ----------------
        atp = tc.alloc_tile_pool(name="atp", bufs=1)
        at_t = [atp.tile([128, S], BF16, tag=f"at{i}", name=f"at{i}")
                for i in range(8)]

        # ---------------- Wo tiles (DMA overlaps attention) ----------------
        wop = tc.alloc_tile_pool(name="wop", bufs=1)
        wo_t = [wop.tile([128, S], BF16, tag=f"wo{i}", name=f"wo{i}") for i in range(8)]
        for i in range(8):
            nc.sync.dma_start(wo_t[i][:], wo[i * 128:(i + 1) * 128, :])

        # ---------------- RoPE (neox) on Q and K, in place ----------------
        ptmp = tc.alloc_tile_pool(name="ptmp", bufs=2)
        for pr in range(6):
            a = qk_t[2 * pr]      # first half of head dim
            b = qk_t[2 * pr + 1]  # second half
            t1 = ptmp.tile([128, S], BF16, tag="r1")
            t2 = ptmp.tile([128, S], BF16, tag="r2")
            nc.vector.tensor_mul(t1[:], b[:], sin_t[:])   # x2*sin
            nc.vector.tensor_mul(t2[:], b[:], cos_t[:])   # x2*cos
            nc.vector.tensor_mul(b[:], a[:], sin_t[:])    # x1*sin
            nc.vector.tensor_add(b[:], b[:], t2[:])       # x2' = x2*cos + x1*sin
            nc.vector.tensor_mul(a[:], a[:], cos_t[:])    # x1*cos
            nc.vector.tensor_sub(a[:], a[:], t1[:])       # x1' = x1*cos - x2*sin

        # ---------------- windowed attention ----------------
        # scores in [k, q] layout; P = exp(S/16) masked; attn^T accumulated
        # over k-tiles; softmax denominators via ones-matmul.
        pP = tc.alloc_tile_pool(name="pP", bufs=6)
        pmisc = tc.alloc_tile_pool(name="pmisc", bufs=2)
        pscore = tc.alloc_tile_pool(name="pscore", bufs=3, space="PSUM")
        ppv = tc.alloc_tile_pool(name="ppv", bufs=1, space="PSUM")
        psums = tc.alloc_tile_pool(name="psums", bufs=1, space="PSUM")
        psbc = tc.alloc_tile_pool(name="psbc", bufs=1, space="PSUM")

        units = []
        for lh in range(4):
            for qb in range(N_SB):
                kt_lo = max(0, qb * 4 - 8)
                kt_hi = qb * 4 + 3
                for kt in range(kt_lo, kt_hi + 1):
                    m_lo = max(kt - qb * 4, 0)
                    m_hi = min(kt + 8 - qb * 4, 3)
                    units.append(dict(
                        lh=lh, qb=qb, kt=kt,
                        first=(kt == kt_lo), last=(kt == kt_hi),
                        off=m_lo * 128, width=(m_hi - m_lo + 1) * 128,
                        diag=(kt >= qb * 4), wedge=(kt + 8 <= qb * 4 + 3),
                    ))

        state = {}

        def emit_scores(u):
            lh, qb, kt = u["lh"], u["qb"], u["kt"]
            kv = lh // 2
            ps = pscore.tile([128, 512], F32, tag="s")
            for dd in range(2):
                nc.tensor.matmul(
                    ps[:, 0:u["width"]],
                    qk_t[8 + 2 * kv + dd][:, kt * 128:(kt + 1) * 128],
                    qk_t[2 * lh + dd][:, qb * 512 + u["off"]:
                                      qb * 512 + u["off"] + u["width"]],
                    start=(dd == 0),
                    stop=(dd == 1),
                )
            u["ps"] = ps

        def emit_rest(u):
            lh, qb, kt = u["lh"], u["qb"], u["kt"]
            kv = lh // 2
            w = u["width"]
            if u["first"]:
                state["pv"] = [ppv.tile([128, 512], F32, tag=f"pv{d}", name=f"pv{d}")
                               for d in range(2)]
                state["sums"] = psums.tile([1, 512], F32, tag="sums",
                                           name="sums")

            p_t = pP.tile([128, 512], BF16, tag="p")
            nc.scalar.activation(
                p_t[:, 0:w], u["ps"][:, 0:w],
                mybir.ActivationFunctionType.Exp, scale=float(SCALE),
            )
            if u["diag"]:
                nc.vector.tensor_mul(p_t[:, 0:128], p_t[:, 0:128], diag_t[:])
            if u["wedge"]:
                nc.vector.tensor_mul(p_t[:, w - 128:w], p_t[:, w - 128:w],
                                     wedge_t[:])
            # attn^T accumulation: lhsT = V tile slice, rhs = P
            for d in range(2):
                nc.tensor.matmul(
                    state["pv"][d][:, u["off"]:u["off"] + w],
                    v_t[kt][:, kv * 256 + d * 128:kv * 256 + (d + 1) * 128],
                    p_t[:, 0:w],
                    start=u["first"],
                    stop=u["last"],
                )
            # denominators
            nc.tensor.matmul(
                state["sums"][0:1, u["off"]:u["off"] + w],
                ones_t[:, 0:1],
                p_t[:, 0:w],
                start=u["first"],
                stop=u["last"],
            )
            if u["last"]:
                recip = pmisc.tile([1, 512], F32, tag="recip")
                recipb = pmisc.tile([1, 512], BF16, tag="recipb")
                nc.vector.reciprocal(recip[:], state["sums"][:])
                nc.vector.tensor_copy(recipb[:], recip[:])
                bc_ps = psbc.tile([128, 512], F32, tag="bc")
                nc.tensor.matmul(bc_ps[:], ones_t[0:1, :], recipb[0:1, :])
                bc_sb = pmisc.tile([128, 512], F32, tag="bcs")
                nc.vector.tensor_copy(bc_sb[:], bc_ps[:])
                for d in range(2):
                    nc.vector.tensor_mul(
                        at_t[2 * lh + d][:, qb * 512:(qb + 1) * 512],
                        state["pv"][d][:], bc_sb[:],
                    )

        # software-pipeline by one unit so PE never waits on ACT exp
        emit_scores(units[0])
        for i in range(1, len(units)):
            emit_scores(units[i])
            emit_rest(units[i - 1])
        emit_rest(units[-1])

        psbc.release()
        psums.release()
        ppv.release()
        pscore.release()

        # ---------------- out^T = Wo^T @ attn^T ----------------
        pso = tc.alloc_tile_pool(name="pso", bufs=4, space="PSUM")
        outst = tc.alloc_tile_pool(name="outst", bufs=3)
        for ot in range(16):
            for qb in range(N_SB):
                ps = pso.tile([128, 512], F32, tag="o")
                for dt in range(8):
                    nc.tensor.matmul(
                        ps[:],
                        wo_t[dt][:, ot * 128:(ot + 1) * 128],
                        at_t[dt][:, qb * 512:(qb + 1) * 512],
                        start=(dt == 0),
                        stop=(dt == 7),
                    )
                o_sb = outst.tile([128, 512], BF16, tag="os")
                nc.vector.tensor_copy(o_sb[:], ps[:])
                nc.sync.dma_start(
                    outT[ot * 128:(ot + 1) * 128, qb * 512:(qb + 1) * 512],
                    o_sb[:],
                )
        pso.release()
        outst.release()
        pmisc.release()
        pP.release()
        ptmp.release()
        wop.release()
        atp.release()
        pp.release()

    nc.compile()
    return nc


def _prep_inputs(hidden_states, positions, Wq, bq, Wk, bk, Wv, bv, Wo):
    hs = np.asarray(hidden_states, dtype=np.float32)
    positions = np.asarray(positions)
    Wq = np.asarray(Wq, dtype=np.float32)
    Wk = np.asarray(Wk, dtype=np.float32)
    Wv = np.asarray(Wv, dtype=np.float32)
    Wo = np.asarray(Wo, dtype=np.float32)
    bq = np.asarray(bq, dtype=np.float32)
    bk = np.asarray(bk, dtype=np.float32)
    bv = np.asarray(bv, dtype=np.float32)

    half = HD // 2
    inv_freq = (1.0 / (ROPE_BASE ** (np.arange(half, dtype=np.float64) / half)))

    k = np.arange(128)[:, None]
    q = np.arange(128)[None, :]
    diag = (k <= q).astype(NPBF16)
    wedge = (k > q).astype(NPBF16)
    ones = np.ones((128, 128), dtype=NPBF16)

    in_maps = []
    for core in range(N_CORES):
        b, hg = core // 2, core % 2
        hsT = np.ascontiguousarray(hs[b].T).astype(NPBF16)
        wq_s = Wq[:, hg * 1024:(hg + 1) * 1024]
        wk_s = Wk[:, hg * 512:(hg + 1) * 512]
        wqk = np.ascontiguousarray(
            np.concatenate([wq_s, wk_s], axis=1)).astype(NPBF16)
        wv_s = np.ascontiguousarray(
            Wv[:, hg * 512:(hg + 1) * 512]).astype(NPBF16)
        wo_s = np.ascontiguousarray(
            Wo[hg * 1024:(hg + 1) * 1024, :]).astype(NPBF16)

        ang = inv_freq[:, None] * positions[b][None, :].astype(np.float64)
        cos = np.cos(ang).astype(NPBF16)
        sin = np.sin(ang).astype(NPBF16)

        bqk = np.concatenate([bq[hg * 1024:(hg + 1) * 1024],
                              bk[hg * 512:(hg + 1) * 512]])
        biasqk = np.ascontiguousarray(
            bqk.reshape(NT_QK, 128).T).astype(np.float32)
        bvb = np.broadcast_to(
            bv[hg * 512:(hg + 1) * 512][None, :], (128, V_D))
        bvb = np.ascontiguousarray(bvb).astype(np.float32)

        in_maps.append(dict(
            hsT=hsT, wqk=wqk, wv=wv_s, wo=wo_s, cosd=cos, sind=sin,
            diagd=diag, wedged=wedge, onesd=ones, biasqkd=biasqk, bvbd=bvb,
        ))
    return in_maps


LAST_EXEC_NS = None
LAST_RESULT = None


def kernel(hidden_states, positions, Wq, bq, Wk, bk, Wv, bv, Wo):
    global LAST_EXEC_NS, LAST_RESULT
    if "nc" not in _CACHE:
        _CACHE["nc"] = _build()
    nc = _CACHE["nc"]
    in_maps = _prep_inputs(hidden_states, positions, Wq, bq, Wk, bk, Wv, bv, Wo)
    res = bass_utils.run_bass_kernel_spmd(nc, in_maps, list(range(N_CORES)))
    LAST_RESULT = res
    LAST_EXEC_NS = res.exec_time_ns
    out = np.empty((B, S, H), dtype=np.float32)
    for b in range(B):
        p0 = res.results[2 * b]["outT"].astype(np.float32)
        p1 = res.results[2 * b + 1]["outT"].astype(np.float32)
        out[b] = (p0 + p1).T
    return out


# revision 21
# speedup vs baseline: 1.0524x; 1.0524x over previous
import numpy as np
import ml_dtypes

from concourse import bass
from concourse import bacc
from concourse import mybir
from concourse.tile import TileContext
from concourse import bass_utils

# Gemma2 sliding-window attention — hardcoded problem shapes.
B, S, H = 4, 2048, 2048
NH, NKV, HD = 8, 4, 256
WINDOW = 1024
ROPE_BASE = 10000.0
SCALE = 256.0 ** -0.5

BF16 = mybir.dt.bfloat16
F32 = mybir.dt.float32
NPBF16 = ml_dtypes.bfloat16

N_CORES = 8
NT_H = H // 128          # 16 h-tiles
NT_S = S // 128          # 16 s-tiles
N_SB = S // 512          # 4 s-blocks
QK_D = 1024 + 512        # per-core Q (4 heads) + K (2 kv heads) transposed rows
NT_QK = QK_D // 128      # 12
V_D = 512                # per-core V columns (2 kv heads)
WO_D = 1024              # per-core Wo contraction rows

_CACHE = {}


def _build():
    nc = bacc.Bacc("TRN2", target_bir_lowering=False, debug=False)

    hsT = nc.dram_tensor("hsT", [H, S], BF16, kind="ExternalInput")
    wqk = nc.dram_tensor("wqk", [H, QK_D], BF16, kind="ExternalInput")
    wv = nc.dram_tensor("wv", [H, V_D], BF16, kind="ExternalInput")
    wo = nc.dram_tensor("wo", [WO_D, H], BF16, kind="ExternalInput")
    cosd = nc.dram_tensor("cosd", [128, S], BF16, kind="ExternalInput")
    sind = nc.dram_tensor("sind", [128, S], BF16, kind="ExternalInput")
    diagd = nc.dram_tensor("diagd", [128, 128], BF16, kind="ExternalInput")
    wedged = nc.dram_tensor("wedged", [128, 128], BF16, kind="ExternalInput")
    onesd = nc.dram_tensor("onesd", [128, 128], BF16, kind="ExternalInput")
    biasqkd = nc.dram_tensor("biasqkd", [128, NT_QK], F32, kind="ExternalInput")
    bvbd = nc.dram_tensor("bvbd", [128, V_D], F32, kind="ExternalInput")
    outT = nc.dram_tensor("outT", [H, S], BF16, kind="ExternalOutput")

    with TileContext(nc) as tc:
        # ---------------- persistent SBUF ----------------
        pp = tc.alloc_tile_pool(name="persist", bufs=1)
        cos_t = pp.tile([128, S], BF16, tag="cos")
        sin_t = pp.tile([128, S], BF16, tag="sin")
        diag_t = pp.tile([128, 128], BF16, tag="diag")
        wedge_t = pp.tile([128, 128], BF16, tag="wedge")
        ones_t = pp.tile([128, 128], BF16, tag="ones")
        biasqk_t = pp.tile([128, NT_QK], F32, tag="biasqk")
        bvb_t = pp.tile([128, V_D], F32, tag="bvb")
        qk_t = [pp.tile([128, S], BF16, tag=f"qk{i}", name=f"qk{i}") for i in range(NT_QK)]
        v_t = [pp.tile([128, V_D], BF16, tag=f"v{i}", name=f"v{i}") for i in range(NT_S)]

        # SWDGE (single queue) for tensors consumed by DVE — a HWDGE
        # transfer fans out across HW queues and the first consuming
        # TensorTensor instruction would need one sync-wait per queue,
        # overflowing the ISA wait capacity.
        nc.gpsimd.dma_start(cos_t[:], cosd[:, :])
        nc.gpsimd.dma_start(sin_t[:], sind[:, :])
        nc.gpsimd.dma_start(diag_t[:], diagd[:, :])
        nc.gpsimd.dma_start(wedge_t[:], wedged[:, :])
        nc.gpsimd.dma_start(ones_t[:], onesd[:, :])
        nc.gpsimd.dma_start(biasqk_t[:], biasqkd[:, :])
        nc.gpsimd.dma_start(bvb_t[:], bvbd[:, :])

        # ---------------- load hidden states (transposed) + weights ----------------
        # wqkp sits below wvp on the stack so wqk DMAs have no zone dep on
        # the V phase; hst/wv DMAs interleaved so V-gemm can start early.
        hsp = tc.alloc_tile_pool(name="hst", bufs=1)
        hst = [hsp.tile([128, S], BF16, tag=f"h{i}", name=f"h{i}") for i in range(NT_H)]
        wqkp = tc.alloc_tile_pool(name="wqkp", bufs=1)
        wqk_t = [wqkp.tile([128, QK_D], BF16, tag=f"wqk{i}", name=f"wqk{i}") for i in range(NT_H)]
        wvp = tc.alloc_tile_pool(name="wvp", bufs=1)
        wv_t = [wvp.tile([128, V_D], BF16, tag=f"wv{i}", name=f"wv{i}") for i in range(NT_H)]
        for i in range(NT_H):
            nc.sync.dma_start(hst[i][:], hsT[i * 128:(i + 1) * 128, :])
            nc.sync.dma_start(wv_t[i][:], wv[i * 128:(i + 1) * 128, :])
        for i in range(NT_H):
            nc.sync.dma_start(wqk_t[i][:], wqk[i * 128:(i + 1) * 128, :])

        # ---------------- V = hs @ Wv  (layout [s, d]) ----------------
        psv = tc.alloc_tile_pool(name="psv", bufs=4, space="PSUM")
        for st in range(NT_S):
            ps = psv.tile([128, V_D], F32, tag="psv")
            for ht in range(NT_H):
                nc.tensor.matmul(
                    ps[:],
                    hst[ht][:, st * 128:(st + 1) * 128],
                    wv_t[ht][:],
                    start=(ht == 0),
                    stop=(ht == NT_H - 1),
                )
            nc.vector.tensor_add(v_t[st][:], ps[:], bvb_t[:])
        psv.release()
        wvp.release()

        # RoPE temps (chunked [128, 512]) — allocated in wv's old zone
        ptmp = tc.alloc_tile_pool(name="ptmp", bufs=4)

        def rope_chunk(pr, sb):
            # in-place neox rotation of one 512-col chunk of pair pr
            a = qk_t[2 * pr][:, sb * 512:(sb + 1) * 512]
            b = qk_t[2 * pr + 1][:, sb * 512:(sb + 1) * 512]
            c = cos_t[:, sb * 512:(sb + 1) * 512]
            s = sin_t[:, sb * 512:(sb + 1) * 512]
            t1 = ptmp.tile([128, 512], BF16, tag="r1", name="r1")
            t2 = ptmp.tile([128, 512], BF16, tag="r2", name="r2")
            nc.vector.tensor_mul(t1[:], b, s)      # x2*sin
            nc.vector.tensor_mul(t2[:], b, c)      # x2*cos
            nc.vector.tensor_mul(b, a, s)          # x1*sin
            nc.vector.tensor_add(b, b, t2[:])      # x2' = x2*cos + x1*sin
            nc.vector.tensor_mul(a, a, c)          # x1*cos
            nc.vector.tensor_sub(a, a, t1[:])      # x1' = x1*cos - x2*sin

        # ---------------- QK^T = (Wqk)^T @ hs^T  (layout [d, s]) ----------------
        # K d-tiles (8..11) first so RoPE'd K is ready before Q of later heads
        # finishes — attention overlaps the tail of this GEMM.
        dt_order = [8, 9, 10, 11, 0, 1, 2, 3, 4, 5, 6, 7]
        psqk = tc.alloc_tile_pool(name="psqk", bufs=6, space="PSUM")
        for dt in dt_order:
            for sb in range(N_SB):
                ps = psqk.tile([128, 512], F32, tag="psqk")
                for ht in range(NT_H):
                    nc.tensor.matmul(
                        ps[:],
                        wqk_t[ht][:, dt * 128:(dt + 1) * 128],
                        hst[ht][:, sb * 512:(sb + 1) * 512],
                        start=(ht == 0),
                        stop=(ht == NT_H - 1),
                    )
                nc.vector.tensor_scalar_add(
                    qk_t[dt][:, sb * 512:(sb + 1) * 512], ps[:],
                    biasqk_t[:, dt:dt + 1],
                )
                if dt % 2 == 1:
                    rope_chunk(dt // 2, sb)
        psqk.release()
        ptmp.release()
        wqkp.release()
        hsp.release()

        # ---------------- attn^T output tiles ----------------
        atp = tc.alloc_tile_pool(name="atp", bufs=1)
        at_t = [atp.tile([128, S], BF16, tag=f"at{i}", name=f"at{i}")
                for i in range(8)]

        # ---------------- Wo tiles (DMA overlaps attention) ----------------
        wop = tc.alloc_tile_pool(name="wop", bufs=1)
        wo_t = [wop.tile([128, S], BF16, tag=f"wo{i}", name=f"wo{i}") for i in range(8)]
        for i in range(8):
            nc.sync.dma_start(wo_t[i][:], wo[i * 128:(i + 1) * 128, :])

        # ---------------- windowed attention ----------------
        # scores in [k, q] layout; P = exp(S/16) masked; attn^T accumulated
        # over k-tiles; softmax denominators via ones-matmul.
        pP = tc.alloc_tile_pool(name="pP", bufs=6)
        pmisc = tc.alloc_tile_pool(name="pmisc", bufs=2)
        pscore = tc.alloc_tile_pool(name="pscore", bufs=3, space="PSUM")
        ppv = tc.alloc_tile_pool(name="ppv", bufs=2, space="PSUM")
        psums = tc.alloc_tile_pool(name="psums", bufs=1, space="PSUM")

        units = []
        for lh in range(4):
            for qb in range(N_SB):
                kt_lo = max(0, qb * 4 - 8)
                kt_hi = qb * 4 + 3
                for kt in range(kt_lo, kt_hi + 1):
                    m_lo = max(kt - qb * 4, 0)
                    m_hi = min(kt + 8 - qb * 4, 3)
                    units.append(dict(
                        lh=lh, qb=qb, kt=kt,
                        first=(kt == kt_lo), last=(kt == kt_hi),
                        off=m_lo * 128, width=(m_hi - m_lo + 1) * 128,
                        diag=(kt >= qb * 4), wedge=(kt + 8 <= qb * 4 + 3),
                    ))

        state = {}

        def emit_scores(u):
            lh, qb, kt = u["lh"], u["qb"], u["kt"]
            kv = lh // 2
            ps = pscore.tile([128, 512], F32, tag="s")
            for dd in range(2):
                nc.tensor.matmul(
                    ps[:, 0:u["width"]],
                    qk_t[8 + 2 * kv + dd][:, kt * 128:(kt + 1) * 128],
                    qk_t[2 * lh + dd][:, qb * 512 + u["off"]:
                                      qb * 512 + u["off"] + u["width"]],
                    start=(dd == 0),
                    stop=(dd == 1),
                )
            u["ps"] = ps

        def emit_rest(u):
            lh, qb, kt = u["lh"], u["qb"], u["kt"]
            kv = lh // 2
            w = u["width"]
            if u["first"]:
                state["pv"] = [ppv.tile([128, 512], F32, tag=f"pv{d}", name=f"pv{d}")
                               for d in range(2)]
                state["sums"] = psums.tile([1, 512], F32, tag="sums",
                                           name="sums")

            p_t = pP.tile([128, 512], BF16, tag="p")
            nc.scalar.activation(
                p_t[:, 0:w], u["ps"][:, 0:w],
                mybir.ActivationFunctionType.Exp, scale=float(SCALE),
            )
            if u["diag"]:
                nc.vector.tensor_mul(p_t[:, 0:128], p_t[:, 0:128], diag_t[:])
            if u["wedge"]:
                nc.vector.tensor_mul(p_t[:, w - 128:w], p_t[:, w - 128:w],
                                     wedge_t[:])
            # attn^T accumulation: lhsT = V tile slice, rhs = P
            for d in range(2):
                nc.tensor.matmul(
                    state["pv"][d][:, u["off"]:u["off"] + w],
                    v_t[kt][:, kv * 256 + d * 128:kv * 256 + (d + 1) * 128],
                    p_t[:, 0:w],
                    start=u["first"],
                    stop=u["last"],
                )
            # denominators
            nc.tensor.matmul(
                state["sums"][0:1, u["off"]:u["off"] + w],
                ones_t[:, 0:1],
                p_t[:, 0:w],
                start=u["first"],
                stop=u["last"],
            )
            if u["last"]:
                # broadcast sums across partitions (PE), then one wide
                # reciprocal on DVE — [1,512] DVE ops are 1-lane and slow.
                recipb = pmisc.tile([1, 512], BF16, tag="recipb")
                nc.scalar.copy(recipb[:], state["sums"][:])
                bc_ps = pscore.tile([128, 512], F32, tag="s", name="bc_ps")
                nc.tensor.matmul(bc_ps[:], ones_t[0:1, :], recipb[0:1, :])
                bc_sb = pmisc.tile([128, 512], F32, tag="bcs")
                nc.vector.reciprocal(bc_sb[:], bc_ps[:])
                for d in range(2):
                    nc.vector.tensor_mul(
                        at_t[2 * lh + d][:, qb * 512:(qb + 1) * 512],
                        state["pv"][d][:], bc_sb[:],
                    )

        # software-pipeline by one unit so PE never waits on ACT exp
        emit_scores(units[0])
        for i in range(1, len(units)):
            emit_scores(units[i])
            emit_rest(units[i - 1])
        emit_rest(units[-1])

        psums.release()
        ppv.release()
        pscore.release()

        # ---------------- out^T = Wo^T @ attn^T ----------------
        pso = tc.alloc_tile_pool(name="pso", bufs=4, space="PSUM")
        outst = tc.alloc_tile_pool(name="outst", bufs=3)
        for ot in range(16):
            for qb in range(N_SB):
                ps = pso.tile([128, 512], F32, tag="o")
                for dt in range(8):
                    nc.tensor.matmul(
                        ps[:],
                        wo_t[dt][:, ot * 128:(ot + 1) * 128],
                        at_t[dt][:, qb * 512:(qb + 1) * 512],
                        start=(dt == 0),
                        stop=(dt == 7),
                    )
                o_sb = outst.tile([128, 512], BF16, tag="os")
                nc.vector.tensor_copy(o_sb[:], ps[:])
                nc.sync.dma_start(
                    outT[ot * 128:(ot + 1) * 128, qb * 512:(qb + 1) * 512],
                    o_sb[:],
                )
        pso.release()
        outst.release()
        pmisc.release()
        pP.release()
        wop.release()
        atp.release()
        pp.release()

    nc.compile()
    return nc


def _prep_inputs(hidden_states, positions, Wq, bq, Wk, bk, Wv, bv, Wo):
    hs = np.asarray(hidden_states, dtype=np.float32)
    positions = np.asarray(positions)
    Wq = np.asarray(Wq, dtype=np.float32)
    Wk = np.asarray(Wk, dtype=np.float32)
    Wv = np.asarray(Wv, dtype=np.float32)
    Wo = np.asarray(Wo, dtype=np.float32)
    bq = np.asarray(bq, dtype=np.float32)
    bk = np.asarray(bk, dtype=np.float32)
    bv = np.asarray(bv, dtype=np.float32)

    half = HD // 2
    inv_freq = (1.0 / (ROPE_BASE ** (np.arange(half, dtype=np.float64) / half)))

    k = np.arange(128)[:, None]
    q = np.arange(128)[None, :]
    diag = (k <= q).astype(NPBF16)
    wedge = (k > q).astype(NPBF16)
    ones = np.ones((128, 128), dtype=NPBF16)

    in_maps = []
    for core in range(N_CORES):
        b, hg = core // 2, core % 2
        hsT = np.ascontiguousarray(hs[b].T).astype(NPBF16)
        wq_s = Wq[:, hg * 1024:(hg + 1) * 1024]
        wk_s = Wk[:, hg * 512:(hg + 1) * 512]
        wqk = np.ascontiguousarray(
            np.concatenate([wq_s, wk_s], axis=1)).astype(NPBF16)
        wv_s = np.ascontiguousarray(
            Wv[:, hg * 512:(hg + 1) * 512]).astype(NPBF16)
        wo_s = np.ascontiguousarray(
            Wo[hg * 1024:(hg + 1) * 1024, :]).astype(NPBF16)

        ang = inv_freq[:, None] * positions[b][None, :].astype(np.float64)
        cos = np.cos(ang).astype(NPBF16)
        sin = np.sin(ang).astype(NPBF16)

        bqk = np.concatenate([bq[hg * 1024:(hg + 1) * 1024],
                              bk[hg * 512:(hg + 1) * 512]])
        biasqk = np.ascontiguousarray(
            bqk.reshape(NT_QK, 128).T).astype(np.float32)
        bvb = np.broadcast_to(
            bv[hg * 512:(hg + 1) * 512][None, :], (128, V_D))
        bvb = np.ascontiguousarray(bvb).astype(np.float32)

        in_maps.append(dict(
            hsT=hsT, wqk=wqk, wv=wv_s, wo=wo_s, cosd=cos, sind=sin,
            diagd=diag, wedged=wedge, onesd=ones, biasqkd=biasqk, bvbd=bvb,
        ))
    return in_maps


LAST_EXEC_NS = None
LAST_RESULT = None


def kernel(hidden_states, positions, Wq, bq, Wk, bk, Wv, bv, Wo):
    global LAST_EXEC_NS, LAST_RESULT
    if "nc" not in _CACHE:
        _CACHE["nc"] = _build()
    nc = _CACHE["nc"]
    in_maps = _prep_inputs(hidden_states, positions, Wq, bq, Wk, bk, Wv, bv, Wo)
    res = bass_utils.run_bass_kernel_spmd(nc, in_maps, list(range(N_CORES)))
    LAST_RESULT = res
    LAST_EXEC_NS = res.exec_time_ns
    out = np.empty((B, S, H), dtype=np.float32)
    for b in range(B):
        p0 = res.results[2 * b]["outT"].astype(np.float32)
        p1 = res.results[2 * b + 1]["outT"].astype(np.float32)
        out[b] = (p0 + p1).T
    return out


# revision 23
# speedup vs baseline: 1.0861x; 1.0321x over previous
import numpy as np
import ml_dtypes

from concourse import bass
from concourse import bacc
from concourse import mybir
from concourse.tile import TileContext
from concourse import bass_utils

# Gemma2 sliding-window attention — hardcoded problem shapes.
B, S, H = 4, 2048, 2048
NH, NKV, HD = 8, 4, 256
WINDOW = 1024
ROPE_BASE = 10000.0
SCALE = 256.0 ** -0.5

BF16 = mybir.dt.bfloat16
F32 = mybir.dt.float32
NPBF16 = ml_dtypes.bfloat16

N_CORES = 8
NT_H = H // 128          # 16 h-tiles
NT_S = S // 128          # 16 s-tiles
N_SB = S // 512          # 4 s-blocks
QK_D = 1024 + 512        # per-core Q (4 heads) + K (2 kv heads) transposed rows
NT_QK = QK_D // 128      # 12
V_D = 512                # per-core V columns (2 kv heads)
WO_D = 1024              # per-core Wo contraction rows

_CACHE = {}


def _build():
    nc = bacc.Bacc("TRN2", target_bir_lowering=False, debug=False)

    hsT = nc.dram_tensor("hsT", [H, S], BF16, kind="ExternalInput")
    wqk = nc.dram_tensor("wqk", [H, QK_D], BF16, kind="ExternalInput")
    wv = nc.dram_tensor("wv", [H, V_D], BF16, kind="ExternalInput")
    wo = nc.dram_tensor("wo", [WO_D, H], BF16, kind="ExternalInput")
    cosd = nc.dram_tensor("cosd", [128, S], BF16, kind="ExternalInput")
    sind = nc.dram_tensor("sind", [128, S], BF16, kind="ExternalInput")
    diagd = nc.dram_tensor("diagd", [128, 128], BF16, kind="ExternalInput")
    wedged = nc.dram_tensor("wedged", [128, 128], BF16, kind="ExternalInput")
    onesd = nc.dram_tensor("onesd", [128, 128], BF16, kind="ExternalInput")
    biasqkd = nc.dram_tensor("biasqkd", [128, NT_QK], F32, kind="ExternalInput")
    bvbd = nc.dram_tensor("bvbd", [128, V_D], F32, kind="ExternalInput")
    outT = nc.dram_tensor("outT", [H, S], BF16, kind="ExternalOutput")

    with TileContext(nc) as tc:
        # ---------------- persistent SBUF ----------------
        pp = tc.alloc_tile_pool(name="persist", bufs=1)
        cos_t = pp.tile([128, S], BF16, tag="cos")
        sin_t = pp.tile([128, S], BF16, tag="sin")
        diag_t = pp.tile([128, 128], BF16, tag="diag")
        wedge_t = pp.tile([128, 128], BF16, tag="wedge")
        ones_t = pp.tile([128, 128], BF16, tag="ones")
        biasqk_t = pp.tile([128, NT_QK], F32, tag="biasqk")
        bvb_t = pp.tile([128, V_D], F32, tag="bvb")
        qk_t = [pp.tile([128, S], BF16, tag=f"qk{i}", name=f"qk{i}") for i in range(NT_QK)]
        v_t = [pp.tile([128, V_D], BF16, tag=f"v{i}", name=f"v{i}") for i in range(NT_S)]

        # SWDGE (single queue) for tensors consumed by DVE — a HWDGE
        # transfer fans out across HW queues and the first consuming
        # TensorTensor instruction would need one sync-wait per queue,
        # overflowing the ISA wait capacity.
        nc.gpsimd.dma_start(cos_t[:], cosd[:, :])
        nc.gpsimd.dma_start(sin_t[:], sind[:, :])
        nc.gpsimd.dma_start(diag_t[:], diagd[:, :])
        nc.gpsimd.dma_start(wedge_t[:], wedged[:, :])
        nc.gpsimd.dma_start(ones_t[:], onesd[:, :])
        nc.gpsimd.dma_start(biasqk_t[:], biasqkd[:, :])
        nc.gpsimd.dma_start(bvb_t[:], bvbd[:, :])

        # ---------------- load hidden states (transposed) + weights ----------------
        # wqkp sits below wvp on the stack so wqk DMAs have no zone dep on
        # the V phase; hst/wv DMAs interleaved so V-gemm can start early.
        hsp = tc.alloc_tile_pool(name="hst", bufs=1)
        hst = [hsp.tile([128, S], BF16, tag=f"h{i}", name=f"h{i}") for i in range(NT_H)]
        wqkp = tc.alloc_tile_pool(name="wqkp", bufs=1)
        wqk_t = [wqkp.tile([128, QK_D], BF16, tag=f"wqk{i}", name=f"wqk{i}") for i in range(NT_H)]
        wvp = tc.alloc_tile_pool(name="wvp", bufs=1)
        wv_t = [wvp.tile([128, V_D], BF16, tag=f"wv{i}", name=f"wv{i}") for i in range(NT_H)]
        for i in range(NT_H):
            nc.sync.dma_start(hst[i][:], hsT[i * 128:(i + 1) * 128, :])
            nc.sync.dma_start(wv_t[i][:], wv[i * 128:(i + 1) * 128, :])
        for i in range(NT_H):
            nc.sync.dma_start(wqk_t[i][:], wqk[i * 128:(i + 1) * 128, :])

        # ---------------- V = hs @ Wv  (layout [s, d]) ----------------
        psv = tc.alloc_tile_pool(name="psv", bufs=4, space="PSUM")
        for st in range(NT_S):
            ps = psv.tile([128, V_D], F32, tag="psv")
            for ht in range(NT_H):
                nc.tensor.matmul(
                    ps[:],
                    hst[ht][:, st * 128:(st + 1) * 128],
                    wv_t[ht][:],
                    start=(ht == 0),
                    stop=(ht == NT_H - 1),
                )
            nc.vector.tensor_add(v_t[st][:], ps[:], bvb_t[:])
        psv.release()
        wvp.release()

        # RoPE temps (chunked [128, 512]) — allocated in wv's old zone
        ptmp = tc.alloc_tile_pool(name="ptmp", bufs=4)

        def rope_chunk(pr, sb):
            # in-place neox rotation of one 512-col chunk of pair pr
            a = qk_t[2 * pr][:, sb * 512:(sb + 1) * 512]
            b = qk_t[2 * pr + 1][:, sb * 512:(sb + 1) * 512]
            c = cos_t[:, sb * 512:(sb + 1) * 512]
            s = sin_t[:, sb * 512:(sb + 1) * 512]
            t1 = ptmp.tile([128, 512], BF16, tag="r1", name="r1")
            t2 = ptmp.tile([128, 512], BF16, tag="r2", name="r2")
            nc.vector.tensor_mul(t1[:], b, s)      # x2*sin
            nc.vector.tensor_mul(t2[:], b, c)      # x2*cos
            nc.vector.tensor_mul(b, a, s)          # x1*sin
            nc.vector.tensor_add(b, b, t2[:])      # x2' = x2*cos + x1*sin
            nc.vector.tensor_mul(a, a, c)          # x1*cos
            nc.vector.tensor_sub(a, a, t1[:])      # x1' = x1*cos - x2*sin

        # ---------------- QK^T = (Wqk)^T @ hs^T  (layout [d, s]) ----------------
        # K d-tiles (8..11) first so RoPE'd K is ready before Q of later heads
        # finishes — attention overlaps the tail of this GEMM.
        dt_order = [8, 9, 10, 11, 0, 1, 2, 3, 4, 5, 6, 7]
        psqk = tc.alloc_tile_pool(name="psqk", bufs=6, space="PSUM")
        for dt in dt_order:
            for sb in range(N_SB):
                ps = psqk.tile([128, 512], F32, tag="psqk")
                for ht in range(NT_H):
                    nc.tensor.matmul(
                        ps[:],
                        wqk_t[ht][:, dt * 128:(dt + 1) * 128],
                        hst[ht][:, sb * 512:(sb + 1) * 512],
                        start=(ht == 0),
                        stop=(ht == NT_H - 1),
                    )
                nc.vector.tensor_scalar_add(
                    qk_t[dt][:, sb * 512:(sb + 1) * 512], ps[:],
                    biasqk_t[:, dt:dt + 1],
                )
                if dt % 2 == 1:
                    rope_chunk(dt // 2, sb)
        psqk.release()
        ptmp.release()
        wqkp.release()
        hsp.release()

        # ---------------- attn^T output tiles ----------------
        atp = tc.alloc_tile_pool(name="atp", bufs=1)
        at_t = [atp.tile([128, S], BF16, tag=f"at{i}", name=f"at{i}")
                for i in range(8)]

        # ---------------- Wo tiles (DMA overlaps attention) ----------------
        wop = tc.alloc_tile_pool(name="wop", bufs=1)
        wo_t = [wop.tile([128, S], BF16, tag=f"wo{i}", name=f"wo{i}") for i in range(8)]
        for i in range(8):
            nc.sync.dma_start(wo_t[i][:], wo[i * 128:(i + 1) * 128, :])

        # ---------------- windowed attention ----------------
        # scores in [k, q] layout; P = exp(S/16) masked; attn^T accumulated
        # over k-tiles; softmax denominators via ones-matmul.
        pP = tc.alloc_tile_pool(name="pP", bufs=6)
        pmisc = tc.alloc_tile_pool(name="pmisc", bufs=2)
        pscore = tc.alloc_tile_pool(name="pscore", bufs=3, space="PSUM")
        ppv = tc.alloc_tile_pool(name="ppv", bufs=2, space="PSUM")
        psums = tc.alloc_tile_pool(name="psums", bufs=1, space="PSUM")

        units = []
        for lh in range(4):
            for qb in range(N_SB):
                kt_lo = max(0, qb * 4 - 8)
                kt_hi = qb * 4 + 3
                for kt in range(kt_lo, kt_hi + 1):
                    m_lo = max(kt - qb * 4, 0)
                    m_hi = min(kt + 8 - qb * 4, 3)
                    units.append(dict(
                        lh=lh, qb=qb, kt=kt,
                        first=(kt == kt_lo), last=(kt == kt_hi),
                        off=m_lo * 128, width=(m_hi - m_lo + 1) * 128,
                        diag=(kt >= qb * 4), wedge=(kt + 8 <= qb * 4 + 3),
                    ))

        state = {}

        def emit_scores(u):
            lh, qb, kt = u["lh"], u["qb"], u["kt"]
            kv = lh // 2
            ps = pscore.tile([128, 512], F32, tag="s")
            for dd in range(2):
                nc.tensor.matmul(
                    ps[:, 0:u["width"]],
                    qk_t[8 + 2 * kv + dd][:, kt * 128:(kt + 1) * 128],
                    qk_t[2 * lh + dd][:, qb * 512 + u["off"]:
                                      qb * 512 + u["off"] + u["width"]],
                    start=(dd == 0),
                    stop=(dd == 1),
                )
            u["ps"] = ps

        def emit_rest(u):
            lh, qb, kt = u["lh"], u["qb"], u["kt"]
            kv = lh // 2
            w = u["width"]
            if u["first"]:
                state["pv"] = [ppv.tile([128, 512], F32, tag=f"pv{d}", name=f"pv{d}")
                               for d in range(2)]
                state["sums"] = psums.tile([1, 512], F32, tag="sums",
                                           name="sums")

            p_t = pP.tile([128, 512], BF16, tag="p")
            nc.scalar.activation(
                p_t[:, 0:w], u["ps"][:, 0:w],
                mybir.ActivationFunctionType.Exp, scale=float(SCALE),
            )
            # masks on GpSimd: DVE is in-order and its eviction muls wait on
            # PE — queueing masks behind them convoys the PV matmuls.
            if u["diag"]:
                nc.gpsimd.tensor_mul(p_t[:, 0:128], p_t[:, 0:128], diag_t[:])
            if u["wedge"]:
                nc.gpsimd.tensor_mul(p_t[:, w - 128:w], p_t[:, w - 128:w],
                                     wedge_t[:])
            # attn^T accumulation: lhsT = V tile slice, rhs = P
            for d in range(2):
                nc.tensor.matmul(
                    state["pv"][d][:, u["off"]:u["off"] + w],
                    v_t[kt][:, kv * 256 + d * 128:kv * 256 + (d + 1) * 128],
                    p_t[:, 0:w],
                    start=u["first"],
                    stop=u["last"],
                )
            # denominators
            nc.tensor.matmul(
                state["sums"][0:1, u["off"]:u["off"] + w],
                ones_t[:, 0:1],
                p_t[:, 0:w],
                start=u["first"],
                stop=u["last"],
            )
            if u["last"]:
                # broadcast sums across partitions (PE), then one wide
                # reciprocal on DVE — [1,512] DVE ops are 1-lane and slow.
                recipb = pmisc.tile([1, 512], BF16, tag="recipb")
                nc.scalar.copy(recipb[:], state["sums"][:])
                bc_ps = pscore.tile([128, 512], F32, tag="s", name="bc_ps")
                nc.tensor.matmul(bc_ps[:], ones_t[0:1, :], recipb[0:1, :])
                bc_sb = pmisc.tile([128, 512], F32, tag="bcs")
                nc.vector.reciprocal(bc_sb[:], bc_ps[:])
                for d in range(2):
                    nc.vector.tensor_mul(
                        at_t[2 * lh + d][:, qb * 512:(qb + 1) * 512],
                        state["pv"][d][:], bc_sb[:],
                    )

        # software-pipeline by one unit so PE never waits on ACT exp
        emit_scores(units[0])
        for i in range(1, len(units)):
            emit_scores(units[i])
            emit_rest(units[i - 1])
        emit_rest(units[-1])

        psums.release()
        ppv.release()
        pscore.release()

        # ---------------- out^T = Wo^T @ attn^T ----------------
        pso = tc.alloc_tile_pool(name="pso", bufs=4, space="PSUM")
        outst = tc.alloc_tile_pool(name="outst", bufs=3)
        for qb in range(N_SB):
            for ot in range(16):
                ps = pso.tile([128, 512], F32, tag="o")
                for dt in range(8):
                    nc.tensor.matmul(
                        ps[:],
                        wo_t[dt][:, ot * 128:(ot + 1) * 128],
                        at_t[dt][:, qb * 512:(qb + 1) * 512],
                        start=(dt == 0),
                        stop=(dt == 7),
                    )
                o_sb = outst.tile([128, 512], BF16, tag="os")
                nc.vector.tensor_copy(o_sb[:], ps[:])
                nc.sync.dma_start(
                    outT[ot * 128:(ot + 1) * 128, qb * 512:(qb + 1) * 512],
                    o_sb[:],
                )
        pso.release()
        outst.release()
        pmisc.release()
        pP.release()
        wop.release()
        atp.release()
        pp.release()

    nc.compile()
    return nc


def _prep_inputs(hidden_states, positions, Wq, bq, Wk, bk, Wv, bv, Wo):
    hs = np.asarray(hidden_states, dtype=np.float32)
    positions = np.asarray(positions)
    Wq = np.asarray(Wq, dtype=np.float32)
    Wk = np.asarray(Wk, dtype=np.float32)
    Wv = np.asarray(Wv, dtype=np.float32)
    Wo = np.asarray(Wo, dtype=np.float32)
    bq = np.asarray(bq, dtype=np.float32)
    bk = np.asarray(bk, dtype=np.float32)
    bv = np.asarray(bv, dtype=np.float32)

    half = HD // 2
    inv_freq = (1.0 / (ROPE_BASE ** (np.arange(half, dtype=np.float64) / half)))

    k = np.arange(128)[:, None]
    q = np.arange(128)[None, :]
    diag = (k <= q).astype(NPBF16)
    wedge = (k > q).astype(NPBF16)
    ones = np.ones((128, 128), dtype=NPBF16)

    in_maps = []
    for core in range(N_CORES):
        b, hg = core // 2, core % 2
        hsT = np.ascontiguousarray(hs[b].T).astype(NPBF16)
        wq_s = Wq[:, hg * 1024:(hg + 1) * 1024]
        wk_s = Wk[:, hg * 512:(hg + 1) * 512]
        wqk = np.ascontiguousarray(
            np.concatenate([wq_s, wk_s], axis=1)).astype(NPBF16)
        wv_s = np.ascontiguousarray(
            Wv[:, hg * 512:(hg + 1) * 512]).astype(NPBF16)
        wo_s = np.ascontiguousarray(
            Wo[hg * 1024:(hg + 1) * 1024, :]).astype(NPBF16)

        ang = inv_freq[:, None] * positions[b][None, :].astype(np.float64)
        cos = np.cos(ang).astype(NPBF16)
        sin = np.sin(ang).astype(NPBF16)

        bqk = np.concatenate([bq[hg * 1024:(hg + 1) * 1024],
                              bk[hg * 512:(hg + 1) * 512]])
        biasqk = np.ascontiguousarray(
            bqk.reshape(NT_QK, 128).T).astype(np.float32)
        bvb = np.broadcast_to(
            bv[hg * 512:(hg + 1) * 512][None, :], (128, V_D))
        bvb = np.ascontiguousarray(bvb).astype(np.float32)

        in_maps.append(dict(
            hsT=hsT, wqk=wqk, wv=wv_s, wo=wo_s, cosd=cos, sind=sin,
            diagd=diag, wedged=wedge, onesd=ones, biasqkd=biasqk, bvbd=bvb,
        ))
    return in_maps


LAST_EXEC_NS = None
LAST_RESULT = None


def kernel(hidden_states, positions, Wq, bq, Wk, bk, Wv, bv, Wo):
    global LAST_EXEC_NS, LAST_RESULT
    if "nc" not in _CACHE:
        _CACHE["nc"] = _build()
    nc = _CACHE["nc"]
    in_maps = _prep_inputs(hidden_states, positions, Wq, bq, Wk, bk, Wv, bv, Wo)
    res = bass_utils.run_bass_kernel_spmd(nc, in_maps, list(range(N_CORES)))
    LAST_RESULT = res
    LAST_EXEC_NS = res.exec_time_ns
    out = np.empty((B, S, H), dtype=np.float32)
    for b in range(B):
        p0 = res.results[2 * b]["outT"].astype(np.float32)
        p1 = res.results[2 * b + 1]["outT"].astype(np.float32)
        out[b] = (p0 + p1).T
    return out
